# revision 17
# baseline (speedup 1.0000x reference)
"""CGConv message-passing kernel for 8 Trainium2 NeuronCores.

Strategy (self-contained; shapes hardcoded for the nn_CGConv problem):
 - Sort edges by destination (col); pad each node's edge list to a
   multiple of 4 so the segment-sum becomes a fixed-stride reduce.
   Pad edges carry padflag=1; a -30 weight on the padflag row drives
   both pre-activations to ~-30, making the pad message exactly 0.
 - Shard nodes into 8 contiguous ranges balanced by padded edge count;
   each core owns its ranges' edges (no collective needed).
 - Streams are fp8e4m3, channel-major, tile-interleaved so one DMA per
   8 tiles covers xrow+xcol (DoubleRow pairs) and one covers attr+padflag.
 - Per 512-edge tile: gate/msg preacts = fp8 DoubleRow matmul (xrow,xcol)
   + fp8 matmul (attr,padflag); the linear bias rides the activation
   engine's per-partition bias operand.
 - sigmoid(g)*softplus(c) = (1+tanh(g/2))/2 * ln(1+e^c): tanh+exp on ACT
   (exp_and_others table), ln batched 16 tiles per instruction
   (natural_log table, 2 table loads per 16 tiles), 2*m = (s+1)*sp as one
   DVE scalar_tensor_tensor, groups-of-4 segment reduce on DVE; the
   factor 1/2 is folded into the host-side merge.
 - Group sums (bf16) go back to DRAM; the host reduces groups to nodes
   (np.add.reduceat), halves them, and adds the residual.
"""

import numpy as np
import ml_dtypes

BF16 = ml_dtypes.bfloat16

N_NODES = 25000
N_EDGES = 400000
C = 128
EC = 64
N_CORES = 8
TILE = 512            # edge slots per tile
GROUP = 4             # edge slots per segment group
GDMA = 8              # tiles per DMA batch
MACRO = 2             # tiles per PSUM/elementwise macro batch
SUPER = 16            # tiles per ln/table superbatch
PADW = -30.0          # padflag weight: drives pad-edge preacts to ~-30


def _f8_dtype():
    import concourse.mybir as mybir
    return mybir.dt.np(mybir.dt.float8e4)


def _prep(x, edge_index, edge_attr, gate_w, gate_b, msg_w, msg_b):
    F8 = _f8_dtype()
    row = np.asarray(edge_index[0]).astype(np.int64)
    col = np.asarray(edge_index[1]).astype(np.int64)
    x = np.asarray(x, dtype=np.float32)
    attr = np.asarray(edge_attr, dtype=np.float32)

    order = np.argsort(col, kind="stable")
    row_s, col_s = row[order], col[order]
    attr8_s = attr[order].astype(F8)

    counts = np.bincount(col_s, minlength=N_NODES)
    pcounts = ((counts + GROUP - 1) // GROUP) * GROUP
    cum = np.cumsum(pcounts)
    total = int(cum[-1])

    # node-range split balancing padded edge counts
    targets = (np.arange(1, N_CORES) * total) // N_CORES
    nb = np.concatenate([[0], np.searchsorted(cum, targets) + 1, [N_NODES]])
    nb = np.maximum.accumulate(nb).astype(np.int64)
    edge_bounds = np.searchsorted(col_s, nb)

    core_pad = [int(pcounts[nb[i]:nb[i + 1]].sum()) for i in range(N_CORES)]
    blk = SUPER * TILE
    e_pad = int(-(-max(core_pad) // blk) * blk)
    n_sup = e_pad // TILE

    x8 = x.astype(F8)

    in_maps = []
    merge_info = []
    for i in range(N_CORES):
        lo, hi = int(nb[i]), int(nb[i + 1])
        sl = slice(int(edge_bounds[i]), int(edge_bounds[i + 1]))
        cnt = counts[lo:hi]
        pcnt = pcounts[lo:hi]
        pstart = np.concatenate([[0], np.cumsum(pcnt)]).astype(np.int64)
        estart = np.concatenate([[0], np.cumsum(cnt)]).astype(np.int64)
        ne = int(estart[-1])
        rank = np.arange(ne, dtype=np.int64) - np.repeat(estart[:-1], cnt)
        slot = np.repeat(pstart[:-1], cnt) + rank

        rowv = np.zeros(e_pad, np.int64)
        rowv[slot] = row_s[sl]
        colv = np.zeros(e_pad, np.int64)
        colv[slot] = col_s[sl]
        pf = np.ones(e_pad, np.float32)
        pf[slot] = 0.0

        attrT = np.zeros((EC + 1, e_pad), dtype=F8)
        attrT[:EC, slot] = attr8_s[sl].T
        attrT[EC] = pf.astype(F8)

        xrT = np.ascontiguousarray(x8[rowv].T)   # [128, e_pad]
        xcT = np.ascontiguousarray(x8[colv].T)
        xr = np.empty((C, n_sup, 2, TILE), dtype=F8)
        xr[:, :, 0, :] = xrT.reshape(C, n_sup, TILE)
        xr[:, :, 1, :] = xcT.reshape(C, n_sup, TILE)

        in_maps.append({
            "xr": np.ascontiguousarray(xr.reshape(C, -1)).view(np.uint8),
            "attr": np.ascontiguousarray(attrT).view(np.uint8),
        })
        merge_info.append((lo, hi, pstart, int(pstart[-1]) // GROUP))

    gw = np.asarray(gate_w, np.float32)
    mw = np.asarray(msg_w, np.float32)
    w12g = np.empty((C, 2, C), dtype=F8)
    w12g[:, 0, :] = gw[:, 0:C].T.astype(F8)
    w12g[:, 1, :] = gw[:, C:2 * C].T.astype(F8)
    w12m = np.empty((C, 2, C), dtype=F8)
    w12m[:, 0, :] = mw[:, 0:C].T.astype(F8)
    w12m[:, 1, :] = mw[:, C:2 * C].T.astype(F8)
    w3g = np.empty((EC + 1, C), dtype=F8)
    w3g[:EC] = gw[:, 2 * C:].T.astype(F8)
    w3g[EC] = F8(PADW)
    w3m = np.empty((EC + 1, C), dtype=F8)
    w3m[:EC] = mw[:, 2 * C:].T.astype(F8)
    w3m[EC] = F8(PADW)

    shared = {
        "w12g": np.ascontiguousarray(w12g.reshape(C, 2 * C)).view(np.uint8),
        "w12m": np.ascontiguousarray(w12m.reshape(C, 2 * C)).view(np.uint8),
        "w3g": np.ascontiguousarray(w3g).view(np.uint8),
        "w3m": np.ascontiguousarray(w3m).view(np.uint8),
        # s = tanh(0.5*g + 0.5*b_gate); t2 = exp(c + b_msg)
        "bg2": (0.5 * np.asarray(gate_b, np.float32)).reshape(C, 1).copy(),
        "bm": np.asarray(msg_b, np.float32).reshape(C, 1).copy(),
    }
    for m in in_maps:
        m.update(shared)

    meta = {"e_pad": e_pad, "n_sup": n_sup}
    return in_maps, meta, merge_info


def _build(meta):
    import concourse.bacc as bacc
    import concourse.mybir as mybir
    from concourse import tile

    n_sup = meta["n_sup"]
    bf = mybir.dt.bfloat16
    f32 = mybir.dt.float32
    u8 = mybir.dt.uint8
    f8 = mybir.dt.float8e4
    AF = mybir.ActivationFunctionType
    ALU = mybir.AluOpType
    DR = mybir.MatmulPerfMode.DoubleRow

    nc = bacc.Bacc(None, target_bir_lowering=False, debug=False)

    xr_d = nc.declare_dram_parameter("xr", [C, n_sup * 2 * TILE], u8, isOutput=False)
    attr_d = nc.declare_dram_parameter("attr", [EC + 1, n_sup * TILE], u8, isOutput=False)
    w12g_d = nc.declare_dram_parameter("w12g", [C, 2 * C], u8, isOutput=False)
    w12m_d = nc.declare_dram_parameter("w12m", [C, 2 * C], u8, isOutput=False)
    w3g_d = nc.declare_dram_parameter("w3g", [EC + 1, C], u8, isOutput=False)
    w3m_d = nc.declare_dram_parameter("w3m", [EC + 1, C], u8, isOutput=False)
    bg2_d = nc.declare_dram_parameter("bg2", [C, 1], f32, isOutput=False)
    bm_d = nc.declare_dram_parameter("bm", [C, 1], f32, isOutput=False)
    gs_d = nc.declare_dram_parameter("gs", [C, n_sup * (TILE // GROUP)], bf, isOutput=True)

    GW = TILE // GROUP            # groups per tile (128)
    MC = MACRO * TILE             # elements per macro (1024)
    NMAC = SUPER // MACRO         # macros per superbatch (8)
    SC = SUPER * TILE             # elements per superbatch (8192)

    with tile.TileContext(nc) as tc:
        with (
            tc.tile_pool(name="const", bufs=1) as cpool,
            tc.tile_pool(name="xrs", bufs=3) as xr_pool,
            tc.tile_pool(name="ats", bufs=3) as at_pool,
            tc.tile_pool(name="sbig", bufs=3) as s_pool,
            tc.tile_pool(name="t2big", bufs=3) as t2_pool,
            tc.tile_pool(name="spbig", bufs=2) as sp_pool,
            tc.tile_pool(name="mbig", bufs=1) as m_pool,
            tc.tile_pool(name="gsout", bufs=3) as gs_pool,
            tc.tile_pool(name="gps", bufs=2, space="PSUM") as gate_pool,
            tc.tile_pool(name="mps", bufs=2, space="PSUM") as msg_pool,
        ):
            w12g_t = cpool.tile([C, 2 * C], u8, tag="w12g")
            nc.scalar.dma_start(w12g_t[:], w12g_d[:])
            w12m_t = cpool.tile([C, 2 * C], u8, tag="w12m")
            nc.scalar.dma_start(w12m_t[:], w12m_d[:])
            w3g_t = cpool.tile([EC + 1, C], u8, tag="w3g")
            nc.scalar.dma_start(w3g_t[:], w3g_d[:])
            w3m_t = cpool.tile([EC + 1, C], u8, tag="w3m")
            nc.scalar.dma_start(w3m_t[:], w3m_d[:])
            bg2_t = cpool.tile([C, 1], f32, tag="bg2")
            nc.scalar.dma_start(bg2_t[:], bg2_d[:])
            bm_t = cpool.tile([C, 1], f32, tag="bm")
            nc.scalar.dma_start(bm_t[:], bm_d[:])

            w12g_ap = w12g_t[:].bitcast(f8).rearrange("p (two m) -> p two m", two=2)
            w12m_ap = w12m_t[:].bitcast(f8).rearrange("p (two m) -> p two m", two=2)
            w3g_ap = w3g_t[:].bitcast(f8)
            w3m_ap = w3m_t[:].bitcast(f8)

            def emit_tail(sb, s_t, t2_t, sp_t, m_t, gs_t, halves=1):
                # sp = ln(1 + t2); 2*m = (s + 1) * sp; groups-of-4 segment
                # reduce; gs out.  The lns stay back-to-back, so the table
                # pattern stays at 2 loads per superbatch.
                HC = SC // halves
                HG = HC // GROUP
                for q in range(halves):
                    qs = slice(q * HC, (q + 1) * HC)
                    nc.scalar.activation(sp_t[:, qs], t2_t[:, qs], AF.Ln, bias=1.0)
                for q in range(halves):
                    qs = slice(q * HC, (q + 1) * HC)
                    gq = slice(q * HG, (q + 1) * HG)
                    nc.vector.scalar_tensor_tensor(m_t[:, qs], s_t[:, qs], 1.0,
                                                   sp_t[:, qs],
                                                   op0=ALU.add, op1=ALU.mult)
                    with nc.allow_low_precision("group sums in bf16"):
                        nc.vector.tensor_reduce(
                            gs_t[:, gq],
                            m_t[:, qs].rearrange("p (g k) -> p g k", k=GROUP),
                            axis=mybir.AxisListType.X, op=ALU.add)
                    nc.gpsimd.dma_start(
                        gs_d[:, sb * (SC // GROUP) + q * HG:
                             sb * (SC // GROUP) + (q + 1) * HG],
                        gs_t[:, gq])

            pending = []
            for sb in range(n_sup // SUPER):
                s_t = s_pool.tile([C, SC], bf, tag="s")
                t2_t = t2_pool.tile([C, SC], bf, tag="t2")
                sp_t = sp_pool.tile([C, SC], bf, tag="sp")
                m_t = m_pool.tile([C, SC], bf, tag="m")
                gs_t = gs_pool.tile([C, SC // GROUP], bf, tag="gs")
                n_macros = 0

                for dd in range(SUPER // GDMA):
                    d = sb * (SUPER // GDMA) + dd
                    xr_t = xr_pool.tile([C, GDMA * 2 * TILE], u8, tag="xr")
                    nc.sync.dma_start(
                        xr_t[:], xr_d[:, d * GDMA * 2 * TILE:(d + 1) * GDMA * 2 * TILE])
                    at_t = at_pool.tile([EC + 1, GDMA * TILE], u8, tag="at")
                    nc.sync.dma_start(
                        at_t[:], attr_d[:, d * GDMA * TILE:(d + 1) * GDMA * TILE])

                    for m in range(GDMA // MACRO):
                        off = (dd * GDMA + m * MACRO) * TILE  # into superbatch tiles
                        gate_ps = gate_pool.tile([C, MC], f32, tag="gate")
                        msg_ps = msg_pool.tile([C, MC], f32, tag="msg")
                        for h in range(MACRO):
                            j = m * MACRO + h            # tile within dma batch
                            xr_ap = xr_t[:, j * 2 * TILE:(j + 1) * 2 * TILE] \
                                .bitcast(f8).rearrange("p (two n) -> p two n", two=2)
                            out = gate_ps[:, h * TILE:(h + 1) * TILE]
                            nc.tensor.matmul(out, w12g_ap, xr_ap,
                                             start=True, stop=False, perf_mode=DR)
                        for h in range(MACRO):
                            j = m * MACRO + h
                            at_ap = at_t[:, j * TILE:(j + 1) * TILE].bitcast(f8)
                            nc.tensor.matmul(gate_ps[:, h * TILE:(h + 1) * TILE],
                                             w3g_ap, at_ap, start=False, stop=True)
                        for h in range(MACRO):
                            j = m * MACRO + h
                            xr_ap = xr_t[:, j * 2 * TILE:(j + 1) * 2 * TILE] \
                                .bitcast(f8).rearrange("p (two n) -> p two n", two=2)
                            nc.tensor.matmul(msg_ps[:, h * TILE:(h + 1) * TILE],
                                             w12m_ap, xr_ap,
                                             start=True, stop=False, perf_mode=DR)
                        for h in range(MACRO):
                            j = m * MACRO + h
                            at_ap = at_t[:, j * TILE:(j + 1) * TILE].bitcast(f8)
                            nc.tensor.matmul(msg_ps[:, h * TILE:(h + 1) * TILE],
                                             w3m_ap, at_ap, start=False, stop=True)

                        # s = tanh(0.5*g + 0.5*bg);  t2 = exp(c + bm)
                        nc.scalar.activation(s_t[:, off:off + MC], gate_ps[:],
                                             AF.Tanh, scale=0.5, bias=bg2_t[:])
                        nc.scalar.activation(t2_t[:, off:off + MC], msg_ps[:],
                                             AF.Exp, scale=1.0, bias=bm_t[:])
                        n_macros += 1
                        # Previous superbatch's ln/STT/reduce tail goes here,
                        # after 2 macros of this superbatch have drained their
                        # PSUM: the PE keeps a 2-macro runway through the ln
                        # window instead of stalling behind it.
                        if n_macros == 2 and len(pending) == 2:
                            emit_tail(*pending[0])
                            emit_tail(*pending[1])
                            pending = []

                pending.append((sb, s_t, t2_t, sp_t, m_t, gs_t))

            # final tails: quarter the last one so the DVE chain and output
            # DMA overlap the last ln instead of serializing the kernel drain
            for p in pending[:-1]:
                emit_tail(*p)
            emit_tail(*pending[-1], halves=4)

    # Pin activation tables: Tanh/Exp resolve to exp_and_others, Ln to
    # natural_log — 2 table loads per superbatch instead of per-op thrash.
    import concourse.bacc as _bacc
    real_get = _bacc.get_activation_tables

    def pinned_tables(arch):
        import concourse.mybir as mybir
        AFt = mybir.ActivationFunctionType
        tabs = real_get(arch)
        out = {}
        for name, funcs in tabs.items():
            if name == "exp_and_others":
                out[name] = {AFt.Exp, AFt.Tanh}
            elif name == "natural_log":
                out[name] = {AFt.Ln}
            else:
                out[name] = set()
        return out

    _bacc.get_activation_tables = pinned_tables
    try:
        nc.compile()
    finally:
        _bacc.get_activation_tables = real_get
    return nc


def _postprocess(x, results, merge_info, meta):
    out = np.asarray(x, np.float32).copy()
    for i in range(N_CORES):
        lo, hi, pstart, n_groups = merge_info[i]
        gs = np.asarray(results[i]["gs"], dtype=np.float32)  # [C, n_sup*GW]
        gsT = np.ascontiguousarray(gs.T)                     # [groups, C]
        pcnt = (pstart[1:] - pstart[:-1])
        sel = pcnt > 0
        if not np.any(sel):
            continue
        starts = (pstart[:-1][sel] // GROUP).astype(np.int64)
        seg = np.add.reduceat(gsT, starts, axis=0)
        out[lo:hi][sel] += 0.5 * seg   # un-fold the (1+s)/2 sigmoid factor
    return out


_CACHE = {}


def kernel(**inputs):
    from concourse.bass_utils import run_bass_kernel_spmd

    in_maps, meta, merge_info = _prep(**inputs)
    key = (meta["e_pad"],)
    if key not in _CACHE:
        _CACHE[key] = _build(meta)
    nc = _CACHE[key]
    res = run_bass_kernel_spmd(nc, in_maps, core_ids=list(range(N_CORES)))
    return _postprocess(inputs["x"], res.results, merge_info, meta)


# revision 19
# speedup vs baseline: 1.0521x; 1.0521x over previous
"""CGConv message-passing kernel for 8 Trainium2 NeuronCores.

Strategy (self-contained; shapes hardcoded for the nn_CGConv problem):
 - Sort edges by destination (col); pad each node's edge list to a
   multiple of 4 so the segment-sum becomes a fixed-stride reduce.
   Pad edges carry padflag=1; a -30 weight on the padflag row drives
   both pre-activations to ~-30, making the pad message exactly 0.
 - Shard nodes into 8 contiguous ranges balanced by padded edge count;
   each core owns its ranges' edges (no collective needed).
 - Streams are fp8e4m3, channel-major, tile-interleaved so one DMA per
   8 tiles covers xrow+xcol (DoubleRow pairs) and one covers attr+padflag.
 - Per 512-edge tile: gate/msg preacts = fp8 DoubleRow matmul (xrow,xcol)
   + fp8 matmul (attr,padflag); the linear bias rides the activation
   engine's per-partition bias operand.
 - sigmoid(g)*softplus(c) = (1+tanh(g/2))/2 * ln(1+e^c): tanh+exp on ACT
   (exp_and_others table), ln batched 16 tiles per instruction
   (natural_log table, 2 table loads per 16 tiles), 2*m = (s+1)*sp as one
   DVE scalar_tensor_tensor, groups-of-4 segment reduce on DVE; the
   factor 1/2 is folded into the host-side merge.
 - Group sums (bf16) go back to DRAM; the host reduces groups to nodes
   (np.add.reduceat), halves them, and adds the residual.
"""

import numpy as np
import ml_dtypes

BF16 = ml_dtypes.bfloat16

N_NODES = 25000
N_EDGES = 400000
C = 128
EC = 64
N_CORES = 8
TILE = 512            # edge slots per tile
GROUP = 4             # edge slots per segment group
GDMA = 8              # tiles per DMA batch
MACRO = 2             # tiles per PSUM/elementwise macro batch
SUPER = 16            # tiles per ln/table superbatch
PADW = -30.0          # padflag weight: drives pad-edge preacts to ~-30


def _f8_dtype():
    import concourse.mybir as mybir
    return mybir.dt.np(mybir.dt.float8e4)


def _prep(x, edge_index, edge_attr, gate_w, gate_b, msg_w, msg_b):
    F8 = _f8_dtype()
    row = np.asarray(edge_index[0]).astype(np.int64)
    col = np.asarray(edge_index[1]).astype(np.int64)
    x = np.asarray(x, dtype=np.float32)
    attr = np.asarray(edge_attr, dtype=np.float32)

    order = np.argsort(col, kind="stable")
    row_s, col_s = row[order], col[order]
    attr8_s = attr[order].astype(F8)

    counts = np.bincount(col_s, minlength=N_NODES)
    pcounts = ((counts + GROUP - 1) // GROUP) * GROUP
    cum = np.cumsum(pcounts)
    total = int(cum[-1])

    # node-range split balancing padded edge counts
    targets = (np.arange(1, N_CORES) * total) // N_CORES
    nb = np.concatenate([[0], np.searchsorted(cum, targets) + 1, [N_NODES]])
    nb = np.maximum.accumulate(nb).astype(np.int64)
    edge_bounds = np.searchsorted(col_s, nb)

    core_pad = [int(pcounts[nb[i]:nb[i + 1]].sum()) for i in range(N_CORES)]
    blk = SUPER * TILE
    e_pad = int(-(-max(core_pad) // blk) * blk)
    n_sup = e_pad // TILE

    x8 = x.astype(F8)

    in_maps = []
    merge_info = []
    for i in range(N_CORES):
        lo, hi = int(nb[i]), int(nb[i + 1])
        sl = slice(int(edge_bounds[i]), int(edge_bounds[i + 1]))
        cnt = counts[lo:hi]
        pcnt = pcounts[lo:hi]
        pstart = np.concatenate([[0], np.cumsum(pcnt)]).astype(np.int64)
        estart = np.concatenate([[0], np.cumsum(cnt)]).astype(np.int64)
        ne = int(estart[-1])
        rank = np.arange(ne, dtype=np.int64) - np.repeat(estart[:-1], cnt)
        slot = np.repeat(pstart[:-1], cnt) + rank

        rowv = np.zeros(e_pad, np.int64)
        rowv[slot] = row_s[sl]
        colv = np.zeros(e_pad, np.int64)
        colv[slot] = col_s[sl]
        pf = np.ones(e_pad, np.float32)
        pf[slot] = 0.0

        attrT = np.zeros((EC + 1, e_pad), dtype=F8)
        attrT[:EC, slot] = attr8_s[sl].T
        attrT[EC] = pf.astype(F8)

        xrT = np.ascontiguousarray(x8[rowv].T)   # [128, e_pad]
        xcT = np.ascontiguousarray(x8[colv].T)
        xr = np.empty((C, n_sup, 2, TILE), dtype=F8)
        xr[:, :, 0, :] = xrT.reshape(C, n_sup, TILE)
        xr[:, :, 1, :] = xcT.reshape(C, n_sup, TILE)

        in_maps.append({
            "xr": np.ascontiguousarray(xr.reshape(C, -1)).view(np.uint8),
            "attr": np.ascontiguousarray(attrT).view(np.uint8),
        })
        merge_info.append((lo, hi, pstart, int(pstart[-1]) // GROUP))

    gw = np.asarray(gate_w, np.float32)
    mw = np.asarray(msg_w, np.float32)
    w12g = np.empty((C, 2, C), dtype=F8)
    w12g[:, 0, :] = gw[:, 0:C].T.astype(F8)
    w12g[:, 1, :] = gw[:, C:2 * C].T.astype(F8)
    w12m = np.empty((C, 2, C), dtype=F8)
    w12m[:, 0, :] = mw[:, 0:C].T.astype(F8)
    w12m[:, 1, :] = mw[:, C:2 * C].T.astype(F8)
    w3g = np.empty((EC + 1, C), dtype=F8)
    w3g[:EC] = gw[:, 2 * C:].T.astype(F8)
    w3g[EC] = F8(PADW)
    w3m = np.empty((EC + 1, C), dtype=F8)
    w3m[:EC] = mw[:, 2 * C:].T.astype(F8)
    w3m[EC] = F8(PADW)

    # one packed weight tensor -> a single startup DMA instead of six
    # s = tanh(0.5*g + 0.5*b_gate); t2 = exp(c + b_msg)
    wpack = np.zeros((C, 776), np.uint8)
    wpack[:, 0:256] = w12g.reshape(C, 2 * C).view(np.uint8)
    wpack[:, 256:512] = w12m.reshape(C, 2 * C).view(np.uint8)
    wpack[:EC + 1, 512:640] = w3g.view(np.uint8)
    wpack[:EC + 1, 640:768] = w3m.view(np.uint8)
    wpack[:, 768:772] = (0.5 * np.asarray(gate_b, np.float32)).reshape(C, 1).view(np.uint8)
    wpack[:, 772:776] = np.asarray(msg_b, np.float32).reshape(C, 1).view(np.uint8)
    shared = {"wpack": wpack}
    for m in in_maps:
        m.update(shared)

    meta = {"e_pad": e_pad, "n_sup": n_sup}
    return in_maps, meta, merge_info


def _build(meta):
    import concourse.bacc as bacc
    import concourse.mybir as mybir
    from concourse import tile

    n_sup = meta["n_sup"]
    bf = mybir.dt.bfloat16
    f32 = mybir.dt.float32
    u8 = mybir.dt.uint8
    f8 = mybir.dt.float8e4
    AF = mybir.ActivationFunctionType
    ALU = mybir.AluOpType
    DR = mybir.MatmulPerfMode.DoubleRow

    nc = bacc.Bacc(None, target_bir_lowering=False, debug=False)

    xr_d = nc.declare_dram_parameter("xr", [C, n_sup * 2 * TILE], u8, isOutput=False)
    attr_d = nc.declare_dram_parameter("attr", [EC + 1, n_sup * TILE], u8, isOutput=False)
    wpack_d = nc.declare_dram_parameter("wpack", [C, 776], u8, isOutput=False)
    gs_d = nc.declare_dram_parameter("gs", [C, n_sup * (TILE // GROUP)], bf, isOutput=True)

    GW = TILE // GROUP            # groups per tile (128)
    MC = MACRO * TILE             # elements per macro (1024)
    NMAC = SUPER // MACRO         # macros per superbatch (8)
    SC = SUPER * TILE             # elements per superbatch (8192)

    with tile.TileContext(nc) as tc:
        with (
            tc.tile_pool(name="const", bufs=1) as cpool,
            tc.tile_pool(name="xrs", bufs=3) as xr_pool,
            tc.tile_pool(name="ats", bufs=3) as at_pool,
            tc.tile_pool(name="sbig", bufs=2) as s_pool,
            tc.tile_pool(name="t2big", bufs=2) as t2_pool,
            tc.tile_pool(name="spbig", bufs=2) as sp_pool,
            tc.tile_pool(name="mbig", bufs=1) as m_pool,
            tc.tile_pool(name="gsout", bufs=3) as gs_pool,
            tc.tile_pool(name="gps", bufs=2, space="PSUM") as gate_pool,
            tc.tile_pool(name="mps", bufs=2, space="PSUM") as msg_pool,
        ):
            wp_t = cpool.tile([C, 776], u8, tag="wpack")
            nc.scalar.dma_start(wp_t[:], wpack_d[:])

            w12g_ap = wp_t[:, 0:256].bitcast(f8).rearrange("p (two m) -> p two m", two=2)
            w12m_ap = wp_t[:, 256:512].bitcast(f8).rearrange("p (two m) -> p two m", two=2)
            w3g_ap = wp_t[:EC + 1, 512:640].bitcast(f8)
            w3m_ap = wp_t[:EC + 1, 640:768].bitcast(f8)
            bg2_t = wp_t[:, 768:772].bitcast(f32)
            bm_t = wp_t[:, 772:776].bitcast(f32)

            def emit_tail(sb, s_t, t2_t, sp_t, m_t, gs_t, halves=1):
                # sp = ln(1 + t2); 2*m = (s + 1) * sp; groups-of-4 segment
                # reduce; gs out.  The lns stay back-to-back, so the table
                # pattern stays at 2 loads per superbatch.
                HC = SC // halves
                HG = HC // GROUP
                for q in range(halves):
                    qs = slice(q * HC, (q + 1) * HC)
                    nc.scalar.activation(sp_t[:, qs], t2_t[:, qs], AF.Ln, bias=1.0)
                for q in range(halves):
                    qs = slice(q * HC, (q + 1) * HC)
                    gq = slice(q * HG, (q + 1) * HG)
                    nc.vector.scalar_tensor_tensor(m_t[:, qs], s_t[:, qs], 1.0,
                                                   sp_t[:, qs],
                                                   op0=ALU.add, op1=ALU.mult)
                    with nc.allow_low_precision("group sums in bf16"):
                        nc.vector.tensor_reduce(
                            gs_t[:, gq],
                            m_t[:, qs].rearrange("p (g k) -> p g k", k=GROUP),
                            axis=mybir.AxisListType.X, op=ALU.add)
                    nc.gpsimd.dma_start(
                        gs_d[:, sb * (SC // GROUP) + q * HG:
                             sb * (SC // GROUP) + (q + 1) * HG],
                        gs_t[:, gq])

            pending = None
            for sb in range(n_sup // SUPER):
                s_t = s_pool.tile([C, SC], bf, tag="s")
                t2_t = t2_pool.tile([C, SC], bf, tag="t2")
                sp_t = sp_pool.tile([C, SC], bf, tag="sp")
                m_t = m_pool.tile([C, SC], bf, tag="m")
                gs_t = gs_pool.tile([C, SC // GROUP], bf, tag="gs")
                n_macros = 0

                for dd in range(SUPER // GDMA):
                    d = sb * (SUPER // GDMA) + dd
                    xr_t = xr_pool.tile([C, GDMA * 2 * TILE], u8, tag="xr")
                    nc.sync.dma_start(
                        xr_t[:], xr_d[:, d * GDMA * 2 * TILE:(d + 1) * GDMA * 2 * TILE])
                    at_t = at_pool.tile([EC + 1, GDMA * TILE], u8, tag="at")
                    nc.sync.dma_start(
                        at_t[:], attr_d[:, d * GDMA * TILE:(d + 1) * GDMA * TILE])

                    for m in range(GDMA // MACRO):
                        off = (dd * GDMA + m * MACRO) * TILE  # into superbatch tiles
                        gate_ps = gate_pool.tile([C, MC], f32, tag="gate")
                        msg_ps = msg_pool.tile([C, MC], f32, tag="msg")
                        for h in range(MACRO):
                            j = m * MACRO + h            # tile within dma batch
                            xr_ap = xr_t[:, j * 2 * TILE:(j + 1) * 2 * TILE] \
                                .bitcast(f8).rearrange("p (two n) -> p two n", two=2)
                            out = gate_ps[:, h * TILE:(h + 1) * TILE]
                            nc.tensor.matmul(out, w12g_ap, xr_ap,
                                             start=True, stop=False, perf_mode=DR)
                        for h in range(MACRO):
                            j = m * MACRO + h
                            at_ap = at_t[:, j * TILE:(j + 1) * TILE].bitcast(f8)
                            nc.tensor.matmul(gate_ps[:, h * TILE:(h + 1) * TILE],
                                             w3g_ap, at_ap, start=False, stop=True)
                        for h in range(MACRO):
                            j = m * MACRO + h
                            xr_ap = xr_t[:, j * 2 * TILE:(j + 1) * 2 * TILE] \
                                .bitcast(f8).rearrange("p (two n) -> p two n", two=2)
                            nc.tensor.matmul(msg_ps[:, h * TILE:(h + 1) * TILE],
                                             w12m_ap, xr_ap,
                                             start=True, stop=False, perf_mode=DR)
                        for h in range(MACRO):
                            j = m * MACRO + h
                            at_ap = at_t[:, j * TILE:(j + 1) * TILE].bitcast(f8)
                            nc.tensor.matmul(msg_ps[:, h * TILE:(h + 1) * TILE],
                                             w3m_ap, at_ap, start=False, stop=True)

                        # s = tanh(0.5*g + 0.5*bg);  t2 = exp(c + bm)
                        nc.scalar.activation(s_t[:, off:off + MC], gate_ps[:],
                                             AF.Tanh, scale=0.5, bias=bg2_t)
                        nc.scalar.activation(t2_t[:, off:off + MC], msg_ps[:],
                                             AF.Exp, scale=1.0, bias=bm_t)
                        n_macros += 1
                        # Previous superbatch's ln/STT/reduce tail goes here,
                        # after 2 macros of this superbatch have drained their
                        # PSUM: the PE keeps a 2-macro runway through the ln
                        # window instead of stalling behind it.
                        if n_macros == 2 and pending is not None:
                            emit_tail(*pending)
                            pending = None

                pending = (sb, s_t, t2_t, sp_t, m_t, gs_t)

            # final tail: halve it so the DVE chain and output DMA overlap
            # the last ln instead of serializing the kernel drain
            emit_tail(*pending, halves=4)

    # Pin activation tables: Tanh/Exp resolve to exp_and_others, Ln to
    # natural_log — 2 table loads per superbatch instead of per-op thrash.
    import concourse.bacc as _bacc
    real_get = _bacc.get_activation_tables

    def pinned_tables(arch):
        import concourse.mybir as mybir
        AFt = mybir.ActivationFunctionType
        tabs = real_get(arch)
        out = {}
        for name, funcs in tabs.items():
            if name == "exp_and_others":
                out[name] = {AFt.Exp, AFt.Tanh}
            elif name == "natural_log":
                out[name] = {AFt.Ln}
            else:
                out[name] = set()
        return out

    _bacc.get_activation_tables = pinned_tables
    try:
        nc.compile()
    finally:
        _bacc.get_activation_tables = real_get
    return nc


def _postprocess(x, results, merge_info, meta):
    out = np.asarray(x, np.float32).copy()
    for i in range(N_CORES):
        lo, hi, pstart, n_groups = merge_info[i]
        gs = np.asarray(results[i]["gs"], dtype=np.float32)  # [C, n_sup*GW]
        gsT = np.ascontiguousarray(gs.T)                     # [groups, C]
        pcnt = (pstart[1:] - pstart[:-1])
        sel = pcnt > 0
        if not np.any(sel):
            continue
        starts = (pstart[:-1][sel] // GROUP).astype(np.int64)
        seg = np.add.reduceat(gsT, starts, axis=0)
        out[lo:hi][sel] += 0.5 * seg   # un-fold the (1+s)/2 sigmoid factor
    return out


_CACHE = {}


def kernel(**inputs):
    from concourse.bass_utils import run_bass_kernel_spmd

    in_maps, meta, merge_info = _prep(**inputs)
    key = (meta["e_pad"],)
    if key not in _CACHE:
        _CACHE[key] = _build(meta)
    nc = _CACHE[key]
    res = run_bass_kernel_spmd(nc, in_maps, core_ids=list(range(N_CORES)))
    return _postprocess(inputs["x"], res.results, merge_info, meta)
